# revision 1
# baseline (speedup 1.0000x reference)
"""CapsLayer2D dynamic-routing kernel for 8x TRN2 NeuronCores — v4.

Shapes (hardcoded):
  inputs: [B=16, R=8, C=8, I=128, DIN=16] fp32
  W:      [K=32, I=128, DIN=16, DOUT=16] fp32
  out:    [B, R, C, K, DOUT] fp32

Math: 3-round dynamic routing, closed form (verified 6e-6 vs reference):
  U[p,k] = res (I x O);  s0 = mean_i U_i;  A = U^T U
  y1 = A s0 = U^T(U s0) ; y2 = A y1
  g = factor(s0); s1 = s0 + g*y1; f = factor(s1)
  out = factor(s2)*s2,  s2 = s0 + (g+f)*y1 + f*g*y2
  factor(s) = (|s|^2/(1+|s|^2)) / sqrt(|s|^2+eps)

Per-core plan (batch sharded across 8 cores, W replicated):
  Host pre-builds Xt[(i%4)*32+d, (i//4)*128+p] and W_r[(i%4)*32+d,
  (i//4)*512+k*16+o] fp16, d padded 16->32 (matmul operands need
  32-aligned partition starts). PE: 32 accumulating matmuls -> s0
  (overlapped with the W DMA); 512 banded 32-deep 128-col matmuls ->
  res fp16 in (g, i, k8, o) order (g = k-group of 8); Scalar engine
  evacuates PSUM. All tiles coexist (no pool reuse), so routing has no
  WAR barrier against production and starts on group 0 immediately.
  Routing per (round, group) in i-halves of 64:
    uv-mul (2x) -> o-tree adds 16->8->4->2 (2x) -> q2-direct (dup pair)
    ut-mul via (oh=8, ol=2) pair views (2x) -> i-tree: adjacent-pair
    level then halving adds, per half; halves summed into y.
  Scratch: A[16KB] mul buffer, B[8KB] first tree level, trees ping-pong
  into dead regions of A. Squash factors g/f/h off the sweep path.
"""

import sys

import numpy as np

sys.path.insert(0, "/opt/trn_rl_repo")

P, I, D, D2, K, O = 128, 128, 16, 32, 32, 16
KC = 8          # k-group size
NG = K // KC    # 4 groups
GN = I * KC * O  # 16384 elements per group block
KO = K * O      # 512
HB = 8192       # half-group block (64 i's)
N_CORES = 8
EPS = 1e-7

_PROGRAM = None


def _build_program():
    from contextlib import ExitStack

    import concourse.tile as tile
    from concourse import bacc, mybir

    F32 = mybir.dt.float32
    F16 = mybir.dt.float16
    ADD = mybir.AluOpType.add
    X = mybir.AxisListType.X
    SQRT = mybir.ActivationFunctionType.Sqrt

    nc = bacc.Bacc("TRN2", target_bir_lowering=False, debug=False)

    xt_d = nc.dram_tensor("xt", [P, 32 * 128], F16, kind="ExternalInput").ap()
    wr_d = nc.dram_tensor("wr", [P, 32 * KO], F16, kind="ExternalInput").ap()
    ei_d = nc.dram_tensor("ei", [P, 256], F16, kind="ExternalInput").ap()
    out_d = nc.dram_tensor("out", [P, KO], F32, kind="ExternalOutput").ap()

    with ExitStack() as ctx:
        tc = ctx.enter_context(tile.TileContext(nc))

        pp = ctx.enter_context(tc.tile_pool(name="pp", bufs=2, space="PSUM"))
        rp = ctx.enter_context(tc.tile_pool(name="resp", bufs=1))
        sm = ctx.enter_context(tc.tile_pool(name="small", bufs=1))

        res = rp.tile([P, NG * GN], F16)     # [P, 65536] (g, i, k8, o)
        UA = rp.tile([P, 4096], F16)         # uv quarter output + tree spots
        UT0 = rp.tile([P, 4096], F16, tag="ut0")
        UT1 = rp.tile([P, 4096], F16, tag="ut1")
        UT = [UT0, UT1]
        q2 = rp.tile([P, I * KC * 2], F16)   # [(i,k8), 2] dup'd uv result
        EI = rp.tile([P, 256], F16)          # [e/64 | e] identities
        Xt = rp.tile([P, 32 * 128], F16)
        Wr = rp.tile([P, 32 * KO], F16)

        # ---- small tiles ----
        s0h = sm.tile([P, KO], F16, tag="s0h")
        y1h = sm.tile([P, KO], F16, tag="y1h")
        y2h = sm.tile([P, KO], F16, tag="y2h")
        sqb = sm.tile([P, KO], F32, tag="sqb")
        s2f = sm.tile([P, KO], F32, tag="s2f")
        yTs = sm.tile([P, KC * O], F16, tag="yTs")
        eps_t = sm.tile([P, 1], F32, tag="eps")
        nc.vector.memset(eps_t[:], EPS)

        def factor(src, out32, tag):
            """out32[p, K] = (nsq/(1+nsq))/sqrt(nsq+eps), nsq over o."""
            nc.scalar.square(sqb[:], src)
            nsq = sm.tile([P, K], F32, tag=f"nsq_{tag}")
            nc.vector.tensor_reduce(
                nsq[:], sqb[:].rearrange("p (k o) -> p k o", k=K), X, ADD
            )
            rt = sm.tile([P, K], F32, tag="f_rt")
            nc.scalar.activation(rt[:], nsq[:], SQRT, bias=eps_t[:])
            b1 = sm.tile([P, K], F32, tag="f_b1")
            nc.scalar.add(b1[:], nsq[:], 1.0)
            den = sm.tile([P, K], F32, tag="f_den")
            nc.vector.tensor_mul(den[:], rt[:], b1[:])
            rin = sm.tile([P, K], F32, tag="f_rin")
            nc.vector.reciprocal(rin[:], den[:])
            nc.vector.tensor_mul(out32[:], nsq[:], rin[:])

        def bcast_o(v32):
            return v32[:].unsqueeze(2).broadcast_to([P, K, O])

        nc.sync.dma_start(EI[:], ei_d)
        # ---- input DMAs (chunked; s0 matmuls chase the Wr pieces) ----
        for q in range(2):
            nc.sync.dma_start(
                Xt[:, q * 2048:(q + 1) * 2048], xt_d[:, q * 2048:(q + 1) * 2048]
            )
        for q in range(8):
            nc.sync.dma_start(
                Wr[:, q * 2048:(q + 1) * 2048], wr_d[:, q * 2048:(q + 1) * 2048]
            )

        # ---- PE warm-up: dummy matmuls on the first Xt piece keep the
        # HAM activity window busy during the Wr DMA so the s0 chain and
        # production run at the warm (2.4 GHz) clock.
        q0 = pp.tile([P, 2048], F32, tag="quad")
        for w in range(24):
            nc.tensor.matmul(
                q0[:, 1024:1152],
                Xt[0:32, 0:128],
                Xt[0:32, 0:128],
                start=(w == 0),
                stop=(w == 23),
                tile_position=(0, 0),
            )

        # ---- s0 = X W / I : 32 accumulating full-depth matmuls ----
        for c in range(32):
            nc.tensor.matmul(
                q0[:, 0:KO],
                Xt[:, c * 128:(c + 1) * 128],
                Wr[:, c * KO:(c + 1) * KO],
                start=(c == 0),
                stop=(c == 31),
            )
        nc.scalar.activation(
            s0h[:], q0[:, 0:KO],
            mybir.ActivationFunctionType.Copy, scale=1.0 / I,
        )
        g32 = sm.tile([P, K], F32, tag="g32")

        # ---- res production: (g, i, k8, o) order ----
        # quad (g, cq): 16 matmuls (band b, chunk 4cq+j) at psum col
        # b*512 + j*128 (bank b <- row-tile bank-conflict rule).
        # Emitted per group, interleaved with round-1 routing below, so
        # the routing accumulation matmuls are not queued behind all of
        # production on the in-order PE.
        def produce_g(g):
            for cq in range(8):
                qt = pp.tile([P, 2048], F32, tag="quad")
                for b in range(4):
                    r0 = b * 32
                    for j in range(4):
                        c = 4 * cq + j
                        nc.tensor.matmul(
                            qt[:, b * 512 + j * 128:b * 512 + (j + 1) * 128],
                            Xt[r0:r0 + 32, c * 128:(c + 1) * 128],
                            Wr[r0:r0 + 32, c * KO + g * 128:c * KO + (g + 1) * 128],
                            start=True,
                            stop=True,
                            tile_position=(r0, 0),
                        )
                # evac: psum (b, j, ko) -> res cols base + j*512 + b*128
                base = g * GN + cq * 2048
                dst = (
                    res[:, base:base + 2048]
                    .rearrange("p (j b o) -> p j b o", j=4, b=4)
                    .transpose([0, 2, 1, 3])
                )
                src = qt[:].rearrange("p (b j o) -> p b j o", b=4, j=4)
                with nc.allow_low_precision(reason="res fp16"):
                    # all evacs on Scalar: the Vector queue must stay clear
                    # so routing starts the moment quad 0 lands.
                    nc.scalar.copy(dst, src)

        # ---- routing ----
        # Per (round, group): DVE does uv-mul + o-tree + q2 + ut-mul in
        # quarter slices of 32 i's; the i-reduction runs on the otherwise
        # idle PE as 128 accumulating "transpose" matmuls (rhs = identity)
        # summing over i into a [ko, p] PSUM tile, which is evacuated and
        # transposed back by one more matmul. Round 1 uses e/64 so y1 is
        # stored pre-scaled (fp16 overflow guard, exact power of two).
        QB = 4096  # quarter block (32 i)

        def route_g(g, v_h16, y_out16, e_accum):
            if True:
                rg = res[:, g * GN:(g + 1) * GN]
                vg = (
                    v_h16[:, g * KC * O:(g + 1) * KC * O]
                    .rearrange("p (k o) -> p k o", k=KC)
                )
                yT = pp.tile([P, 2048], F32, tag="quad")
                for qq in range(4):
                    rq = rg[:, qq * QB:(qq + 1) * QB]
                    # uv mul (32 i's)
                    nc.vector.tensor_mul(
                        UA[:].rearrange("p (i k o) -> p i k o", i=32, k=KC, o=O),
                        rq.rearrange("p (i k o) -> p i k o", i=32, k=KC, o=O),
                        vg.unsqueeze(1).broadcast_to([P, 32, KC, O]),
                    )
                    utq = UT[qq % 2]
                    # o-tree 16->8->4->2 (t8 in utq, t4/t2 in dead UA)
                    tv = UA[:].rearrange("p (ik o) -> p ik o", o=16)
                    t8 = utq[:, 0:2048].rearrange("p (ik o) -> p ik o", o=8)
                    nc.vector.tensor_add(t8, tv[:, :, 0:8], tv[:, :, 8:16])
                    t4 = UA[:, 0:1024].rearrange("p (ik o) -> p ik o", o=4)
                    nc.vector.tensor_add(t4, t8[:, :, 0:4], t8[:, :, 4:8])
                    t2 = UA[:, 1024:1536].rearrange("p (ik o) -> p ik o", o=2)
                    nc.vector.tensor_add(t2, t4[:, :, 0:2], t4[:, :, 2:4])
                    qh = q2[:, qq * 512:(qq + 1) * 512].rearrange(
                        "p (ik j) -> p ik j", j=2
                    )
                    nc.vector.tensor_add(
                        qh,
                        t2[:, :, 0:1].broadcast_to([P, 32 * KC, 2]),
                        t2[:, :, 1:2].broadcast_to([P, 32 * KC, 2]),
                    )
                    # ut mul (pair view oh=8, ol=2) -> utq
                    nc.vector.tensor_mul(
                        utq[:].rearrange("p (ik oh ol) -> p ik oh ol", oh=8, ol=2),
                        rq.rearrange("p (ik oh ol) -> p ik oh ol", oh=8, ol=2),
                        qh.unsqueeze(2).broadcast_to([P, 32 * KC, 8, 2]),
                    )
                    # PE: accumulate sum_i via transposing matmuls
                    for ib in range(32):
                        nc.tensor.matmul(
                            yT[:, 0:128],
                            utq[:, ib * 128:(ib + 1) * 128],
                            e_accum,
                            start=(qq == 0 and ib == 0),
                            stop=(qq == 3 and ib == 31),
                        )
                # evac yT [ko, p] -> sbuf fp16, transpose back, evac y
                with nc.allow_low_precision(reason="y fp16"):
                    nc.scalar.copy(yTs[:], yT[:, 0:128])
                y2p = pp.tile([P, 2048], F32, tag="quad")
                nc.tensor.matmul(
                    y2p[:, 0:128], yTs[:], EI[:, 128:256],
                    start=True, stop=True,
                )
                with nc.allow_low_precision(reason="y fp16"):
                    nc.scalar.copy(
                        y_out16[:, g * KC * O:(g + 1) * KC * O], y2p[:, 0:128]
                    )

        with nc.allow_low_precision(reason="fp16 routing"):
            # round 1: y1/64 = (A s0)/64  (scale via e/64 accumulator),
            # interleaved with per-group production
            SC = 64.0
            produce_g(0)
            produce_g(1)
            route_g(0, s0h, y1h, EI[:, 0:128])
            produce_g(2)
            route_g(1, s0h, y1h, EI[:, 0:128])
            produce_g(3)
            route_g(2, s0h, y1h, EI[:, 0:128])
            route_g(3, s0h, y1h, EI[:, 0:128])
            # g = factor(s0) (kept off the routing-start critical path)
            factor(s0h[:], g32, "g")
            # f = factor(s1), s1 = s0 + g*y1 = s0 + (64 g)*(y1/64)
            g64 = sm.tile([P, K], F32, tag="g64")
            nc.scalar.mul(g64[:], g32[:], SC)
            outf = sm.tile([P, KO], F32, tag="outf")
            s1f = outf
            nc.vector.tensor_mul(
                s1f[:].rearrange("p (k o) -> p k o", k=K),
                y1h[:].rearrange("p (k o) -> p k o", k=K),
                bcast_o(g64),
            )
            nc.vector.tensor_add(s1f[:], s1f[:], s0h[:])
            # f-factor + the y1 part of s2 now, so their Scalar-engine
            # latency hides under round 2 instead of serializing after it.
            f32_ = sm.tile([P, K], F32, tag="f32_")
            factor(s1f[:], f32_, "f")
            # s2 = s0 + (g+f)*y1 + (f*g)*y2
            #    = s0 + 64(g+f)*(y1/64) + 64(f*g)*(y2/64)
            gf = sm.tile([P, K], F32, tag="gf")
            nc.vector.tensor_add(gf[:], g32[:], f32_[:])
            nc.scalar.mul(gf[:], gf[:], SC)
            fg = sm.tile([P, K], F32, tag="fg")
            nc.vector.tensor_mul(fg[:], f32_[:], g32[:])
            nc.scalar.mul(fg[:], fg[:], SC)
            nc.vector.tensor_mul(
                s2f[:].rearrange("p (k o) -> p k o", k=K),
                y1h[:].rearrange("p (k o) -> p k o", k=K),
                bcast_o(gf),
            )
            nc.vector.tensor_add(s2f[:], s2f[:], s0h[:])

            # round 2: y2/64 = A (y1/64)
            for g in range(NG):
                route_g(g, y1h, y2h, EI[:, 128:256])

            nc.vector.tensor_mul(
                sqb[:].rearrange("p (k o) -> p k o", k=K),
                y2h[:].rearrange("p (k o) -> p k o", k=K),
                bcast_o(fg),
            )
            nc.vector.tensor_add(s2f[:], s2f[:], sqb[:])
            # out = factor(s2) * s2
            h32 = sm.tile([P, K], F32, tag="h32")
            factor(s2f[:], h32, "h")
            nc.vector.tensor_mul(
                outf[:].rearrange("p (k o) -> p k o", k=K),
                s2f[:].rearrange("p (k o) -> p k o", k=K),
                bcast_o(h32),
            )
        nc.sync.dma_start(out_d, outf[:])

    nc.compile()
    return nc


def _host_prep(x, W):
    """x: [B,R,C,I,D] f32; W: [K,I,D,O] f32 -> per-core Xt + shared W_r.

    Xt[(i%4)*32+d, (i//4)*128+p] = x[p, i, d] (d < 16, pad to 32).
    W_r[(i%4)*32+d, (i//4)*512+k*16+o] = W[k, i, d, o].
    """
    xs = x.reshape(N_CORES, P, I, D)
    a = xs.transpose(0, 2, 3, 1).reshape(N_CORES, 32, 4, D, P)
    ap = np.zeros((N_CORES, 32, 4, D2, P), np.float32)
    ap[:, :, :, 0:D, :] = a
    xt = (
        ap.transpose(0, 2, 3, 1, 4)
        .reshape(N_CORES, 128, 32 * 128)
        .astype(np.float16)
    )
    b = W.transpose(1, 2, 0, 3).reshape(32, 4, D, KO)
    bp = np.zeros((32, 4, D2, KO), np.float32)
    bp[:, :, 0:D, :] = b
    wr = bp.transpose(1, 2, 0, 3).reshape(128, 32 * KO).astype(np.float16)
    return xt, wr


def _get_program():
    global _PROGRAM
    if _PROGRAM is None:
        _PROGRAM = _build_program()
    return _PROGRAM


def kernel(**inputs):
    x = np.ascontiguousarray(np.asarray(inputs["inputs"], dtype=np.float32))
    W = np.ascontiguousarray(np.asarray(inputs["W"], dtype=np.float32))
    assert x.shape == (16, 8, 8, 128, 16) and W.shape == (32, 128, 16, 16)

    from concourse.bass_utils import run_bass_kernel_spmd

    nc = _get_program()
    xt, wr = _host_prep(x, W)
    e = np.eye(128, dtype=np.float16)
    ei = np.concatenate([e / 64.0, e], axis=1).astype(np.float16)
    in_maps = [
        {"xt": np.ascontiguousarray(xt[c]), "wr": wr, "ei": ei}
        for c in range(N_CORES)
    ]
    r = run_bass_kernel_spmd(nc, in_maps, list(range(N_CORES)))
    outs = [r.results[c]["out"].reshape(2, 8, 8, K, O) for c in range(N_CORES)]
    return np.concatenate(outs, axis=0).astype(np.float32)

